# revision 16
# baseline (speedup 1.0000x reference)
"""Causal multi-head attention (S=2048, B=2, D=2048, H=16, dh=128) on 8 TRN2
NeuronCores.

Sharding: tensor-parallel by heads. Core c owns heads {2c, 2c+1}: it projects
q/k/v for those heads from the full x, applies RoPE, runs causal attention,
then an AllToAll re-shards the context from head-split to token-split and each
core computes its 512-token slice of the output projection. Host-side work is
layout only: transposes, per-head even/odd permutation of W_q/W_k rows (so the
RoPE pair-swap becomes a 64-partition block swap), cos/sin tables from freqs,
causal mask tiles, and the final concat of per-core token slices.

Compute is bf16 (f32 PSUM accumulation); softmax runs without max-subtraction
(scores are O(1) by construction: x ~ N(0,1), W ~ N(0, 1/D)).
"""
import numpy as np
import ml_dtypes

import concourse.bass as bass
import concourse.bacc as bacc
import concourse.mybir as mybir
import concourse.tile as tile

N_CORES = 8
D_MODEL = 2048
N_HEADS = 16
D_HEAD = 128
SEQ = 2048
BATCH = 2
T = SEQ * BATCH          # 4096 tokens, batch-major: t = b*SEQ + s
H_PER_CORE = 2           # heads per core
E_QK = 512               # q+k features per core (2 heads x 128 x 2)
E_V = 256                # v features per core
N_DT = D_MODEL // 128    # 16 d-tiles
N_CHUNK = T // 512       # 8 token chunks of 512
SQ_CHUNKS = SEQ // 512   # 4 query chunks per (b,h) pair
SK_TILES = SEQ // 128    # 16 key tiles per (b,h) pair
T_SLICE = T // N_CORES   # 512 tokens per core in the output phase

F32 = mybir.dt.float32
BF16 = mybir.dt.bfloat16
BF16_NP = ml_dtypes.bfloat16

_cached = {}


def build():
    nc = bacc.Bacc("TRN2", target_bir_lowering=False, debug=False, num_devices=N_CORES)

    xt_ext = nc.declare_dram_parameter("xt", [D_MODEL, T], BF16, isOutput=False)
    wqk_ext = nc.declare_dram_parameter("wqk", [D_MODEL, E_QK], BF16, isOutput=False)
    bqk_ext = nc.declare_dram_parameter("bqk", [4, 128], F32, isOutput=False)
    wv_ext = nc.declare_dram_parameter("wv", [D_MODEL, E_V], BF16, isOutput=False)
    bv_ext = nc.declare_dram_parameter("bv", [2, 128], F32, isOutput=False)
    id_ext = nc.declare_dram_parameter("ident", [128, 128], BF16, isOutput=False)
    cos_ext = nc.declare_dram_parameter("cosT", [2, 128, SEQ], BF16, isOutput=False)
    sin_ext = nc.declare_dram_parameter("sinT", [2, 128, SEQ], BF16, isOutput=False)
    mask_ext = nc.declare_dram_parameter("masks", [4, 128, 512], BF16, isOutput=False)
    wo_ext = nc.declare_dram_parameter("wo", [D_MODEL, D_MODEL], BF16, isOutput=False)
    bo_ext = nc.declare_dram_parameter("bo", [1, D_MODEL], BF16, isOutput=False)
    out_ext = nc.declare_dram_parameter("out", [T_SLICE, D_MODEL], F32, isOutput=True)

    with tile.TileContext(nc) as tc:
        _body(nc, tc, xt_ext, wqk_ext, bqk_ext, wv_ext, bv_ext, id_ext, cos_ext, sin_ext,
              mask_ext, wo_ext, bo_ext, out_ext)
    nc.compile()
    return nc


def _body(nc, tc, xt_ext, wqk_ext, bqk_ext, wv_ext, bv_ext, id_ext, cos_ext, sin_ext,
          mask_ext, wo_ext, bo_ext, out_ext):
    EXP = mybir.ActivationFunctionType.Exp
    IDENT = mybir.ActivationFunctionType.Identity

    with tc.tile_pool(name="res", bufs=1) as res, \
         tc.tile_pool(name="dram", bufs=1, space="DRAM") as dram:
        # ---- resident tiles -------------------------------------------------
        wqk = res.tile([128, N_DT, E_QK], BF16, tag="wqk")
        wv = res.tile([128, N_DT, E_V], BF16, tag="wv")
        masks = res.tile([128, 4, 512], BF16, tag="masks")
        bqk = res.tile([128, 4], F32, tag="bqk")
        bv = res.tile([128, 2], F32, tag="bv")
        ident = res.tile([128, 128], BF16, tag="ident")
        bo = res.tile([1, D_MODEL], BF16, tag="bo")
        ones128 = res.tile([128, 1], BF16, tag="ones128")
        ones1 = res.tile([1, 128], F32, tag="ones1")
        onesb = res.tile([1, 128], BF16, tag="onesb")

        q = res.tile([128, 4, SEQ], BF16, tag="q")     # [dh, pair, s]
        k = res.tile([128, 4, SEQ], BF16, tag="k")
        v = res.tile([128, 4, SK_TILES, D_HEAD], BF16, tag="v")  # [sk_in_tile, pair, sk_tile, dh]

        a2a_in = [dram.tile([8, 128, 512], BF16, name=f"a2a_in{i}", tag=f"a2a_in{i}") for i in range(2)]
        a2a_out = [dram.tile([8, 128, 512], BF16, name=f"a2a_out{i}", tag=f"a2a_out{i}") for i in range(2)]

        nc.scalar.dma_start(out=wqk[:], in_=wqk_ext[:, :].rearrange("(n p) e -> p n e", p=128))
        nc.scalar.dma_start(out=wv[:], in_=wv_ext[:, :].rearrange("(n p) e -> p n e", p=128))
        nc.scalar.dma_start(out=masks[:], in_=mask_ext[:, :, :].rearrange("m p f -> p m f"))
        nc.scalar.dma_start(out=bqk[:], in_=bqk_ext[:, :].rearrange("e p -> p e"))
        nc.scalar.dma_start(out=bv[:], in_=bv_ext[:, :].rearrange("e p -> p e"))
        nc.scalar.dma_start(out=ident[:], in_=id_ext[:, :])
        nc.scalar.dma_start(out=bo[:], in_=bo_ext[:, :])
        nc.gpsimd.memset(ones128[:], 1.0)
        nc.gpsimd.memset(ones1[:], 1.0)
        nc.gpsimd.memset(onesb[:], 1.0)

        # ---- phase 1: QKV projection + RoPE (1024-wide token chunks) -------
        with tc.tile_pool(name="p1sb", bufs=2) as p1sb, \
             tc.tile_pool(name="p1tab", bufs=1) as p1tab, \
             tc.tile_pool(name="p1tmp", bufs=4) as p1tmp, \
             tc.tile_pool(name="p1ps", bufs=3, space="PSUM") as p1ps, \
             tc.tile_pool(name="p1tp", bufs=2, space="PSUM") as p1tp:
            cosT = p1tab.tile([128, 2, SEQ], BF16, tag="cosT")   # [part, qk, s]
            sinT = p1tab.tile([128, 2, SEQ], BF16, tag="sinT")
            nc.scalar.dma_start(out=cosT[:], in_=cos_ext[:, :, :].rearrange("i p s -> p i s"))
            nc.scalar.dma_start(out=sinT[:], in_=sin_ext[:, :, :].rearrange("i p s -> p i s"))
            for ch in range(4):
                b, half = divmod(ch, 2)
                s0 = half * 1024
                xc = p1sb.tile([128, N_DT, 1024], BF16, tag="xc")
                for qd in range(4):
                    nc.sync.dma_start(
                        out=xc[:, qd * 4:(qd + 1) * 4, :],
                        in_=xt_ext[qd * 512:(qd + 1) * 512, ch * 1024:(ch + 1) * 1024]
                        .rearrange("(n p) t -> p n t", p=128))

                # q/k/v^T: psum[e,128 x t,1024] accumulated over 16 d-tiles
                # et 0,1: q h0,h1; 2,3: k h0,h1; 4,5: v h0,h1 (transposed after)
                for et in range(6):
                    ps = p1ps.tile([128, 1024], F32, tag="qk_ps")
                    for u in range(2):
                        for dt in range(N_DT):
                            if et < 4:
                                lhsT = wqk[:, dt, et * 128:(et + 1) * 128]
                            else:
                                lhsT = wv[:, dt, (et - 4) * 128:(et - 3) * 128]
                            nc.tensor.matmul(ps[:, u * 512:(u + 1) * 512], lhsT=lhsT,
                                             rhs=xc[:, dt, u * 512:(u + 1) * 512],
                                             start=(dt == 0), stop=(dt == N_DT - 1))
                    raw = p1tmp.tile([128, 1024], BF16, tag="qkraw")
                    if et < 4:
                        nc.scalar.activation(raw[:], ps[:], IDENT, bias=bqk[:, et:et + 1])
                        # RoPE: dest = raw*cos + blockswap(raw)*sin (sin pre-signed/swapped)
                        qk_i = 0 if et < 2 else 1        # q tables / k tables (scaled)
                        pair = (et % 2) * 2 + b
                        dest = (q if et < 2 else k)[:, pair, s0:s0 + 1024]
                        cs = cosT[:, qk_i, s0:s0 + 1024]
                        sn = sinT[:, qk_i, s0:s0 + 1024]
                        tmp = p1tmp.tile([128, 1024], BF16, tag="ropetmp")
                        nc.vector.tensor_mul(dest, raw[:], cs)
                        nc.vector.tensor_mul(tmp[0:64, :], raw[64:128, :], sn[64:128, :])
                        nc.vector.tensor_mul(tmp[64:128, :], raw[0:64, :], sn[0:64, :])
                        nc.vector.tensor_add(dest, dest, tmp[:])
                    else:
                        hv = et - 4
                        nc.scalar.activation(raw[:], ps[:], IDENT, bias=bv[:, hv:hv + 1])
                        pair = hv * 2 + b
                        for blk in range(8):
                            tp = p1tp.tile([128, 128], BF16, tag="tp")
                            nc.tensor.transpose(tp[:], raw[:, blk * 128:(blk + 1) * 128],
                                                ident[:])
                            nc.vector.tensor_copy(v[:, pair, half * 8 + blk, :], tp[:])

        # ---- phases 2+3 ----------------------------------------------------
        with tc.tile_pool(name="late", bufs=1) as late:
            wo = late.tile([128, N_DT, D_MODEL], BF16, tag="wo")
            ctxg = [late.tile([128, 8, 512], BF16, name=f"ctxg{i}", tag=f"ctxg{i}") for i in range(2)]
            nc.scalar.dma_start(out=wo[:], in_=wo_ext[:, :].rearrange("(n p) e -> p n e", p=128))
            _phase23(nc, tc, q, k, v, masks, ones128, ones1, onesb, bo, wo, ctxg,
                     a2a_in, a2a_out, out_ext)


def _phase23(nc, tc, q, k, v, masks, ones128, ones1, onesb, bo, wo, ctxg,
             a2a_in, a2a_out, out_ext):
        EXP = mybir.ActivationFunctionType.Exp
        # ---- phase 2: causal attention, per (head, batch) pair -------------
        with tc.tile_pool(name="p2exp", bufs=6) as p2exp, \
             tc.tile_pool(name="p2tmp", bufs=3) as p2tmp, \
             tc.tile_pool(name="p2dt", bufs=6) as p2dt, \
             tc.tile_pool(name="p2ps", bufs=2, space="PSUM") as p2ps, \
             tc.tile_pool(name="p2ctx", bufs=2, space="PSUM") as p2ctx, \
             tc.tile_pool(name="p2dn", bufs=2, space="PSUM") as p2dn:
            for h in range(2):                   # local head; A2A #h after its 2 pairs
                for j in range(SQ_CHUNKS):
                    sq0 = j * 512
                    n_sk = 4 * (j + 1)
                    ctx_ps = {}
                    dn_ps = {}
                    for b in range(BATCH):
                        ctx_ps[b] = p2ctx.tile([128, 512], F32, name=f"ctx_ps{b}", tag="ctx_ps")
                        dn_ps[b] = p2dn.tile([1, 512], F32, name=f"dn_ps{b}", tag="dnbc")
                    prev_ex = {}
                    for g in range(n_sk // 2):       # groups of 2 sk-tiles
                        for b in range(BATCH):       # interleave the two batches
                            pair = h * 2 + b
                            sc_ps = p2ps.tile([128, 1024], F32, name=f"sc_ps{b}", tag="sc_ps")
                            for u in range(2):
                                i = 2 * g + u
                                nc.tensor.matmul(sc_ps[:, u * 512:(u + 1) * 512],
                                                 lhsT=k[:, pair, i * 128:(i + 1) * 128],
                                                 rhs=q[:, pair, sq0:sq0 + 512],
                                                 start=True, stop=True)
                            ex = p2exp.tile([128, 1024], BF16, name=f"ex{b}", tag="ex")
                            nc.scalar.activation(ex[:, 0:512], sc_ps[:, 0:512], EXP)
                            nc.scalar.activation(ex[:, 512:1024], sc_ps[:, 512:1024], EXP)
                            if 2 * g >= 4 * j:           # diagonal group: causal mask
                                m = 2 * g - 4 * j        # 0 or 2
                                nc.vector.tensor_mul(ex[:], ex[:], masks[:, m:m + 2, :]
                                                     .rearrange("p m f -> p (m f)"))
                            for u in range(2):
                                i = 2 * g + u
                                nc.tensor.matmul(ctx_ps[b][:], lhsT=v[:, pair, i, :],
                                                 rhs=ex[:, u * 512:(u + 1) * 512],
                                                 start=(i == 0), stop=(i == n_sk - 1))
                            # denominator: tree-sum 4 sk tiles in bf16, then one
                            # ones-matmul per quad into the f32 psum accumulator
                            t1 = p2dt.tile([128, 512], BF16, name=f"t1_{b}", tag="dtree")
                            nc.vector.tensor_add(t1[:], ex[:, 0:512], ex[:, 512:1024])
                            if g % 2 == 0:
                                prev_ex[b] = t1
                            else:
                                t3 = p2dt.tile([128, 512], BF16, name=f"t3_{b}", tag="dtree")
                                nc.vector.tensor_add(t3[:], t1[:], prev_ex[b][:])
                                nc.tensor.matmul(dn_ps[b][:], lhsT=ones128[:], rhs=t3[:],
                                                 start=(g == 1), stop=(g == n_sk // 2 - 1))
                    for b in range(BATCH):
                        # normalize: ctx * (1/denom) broadcast across partitions
                        recip = p2tmp.tile([1, 512], F32, name=f"recip{b}", tag="recip")
                        nc.vector.reciprocal_approx_fast(out=recip[:], in_=dn_ps[b][:])
                        bc_ps = p2dn.tile([128, 512], F32, name=f"bc_ps{b}", tag="dnbc")
                        nc.tensor.matmul(bc_ps[:], lhsT=ones1[:], rhs=recip[:],
                                         start=True, stop=True)
                        bc = p2tmp.tile([128, 512], F32, name=f"bc{b}", tag="bc")
                        nc.scalar.copy(bc[:], bc_ps[:])
                        ctx_sb = p2tmp.tile([128, 512], BF16, name=f"ctx_sb{b}", tag="ctx_sb")
                        nc.vector.tensor_mul(ctx_sb[:], ctx_ps[b][:], bc[:])
                        nc.sync.dma_start(out=a2a_in[h][4 * b + j, :, :], in_=ctx_sb[:])
                nc.gpsimd.collective_compute(
                    "AllToAll", mybir.AluOpType.bypass,
                    replica_groups=[list(range(N_CORES))],
                    ins=[a2a_in[h][:, :, :].opt()],
                    outs=[a2a_out[h][:, :, :].opt()])
                nc.sync.dma_start(out=ctxg[h][:],
                                  in_=a2a_out[h][:, :, :].rearrange("j p t -> p j t"))

        # ---- phase 3: output projection on this core's 512-token slice -----
        # split over the two A2A halves: even-head dims right after A2A#0
        # (overlaps A2A#1), odd-head dims after A2A#1.
        with tc.tile_pool(name="p3sb", bufs=3) as p3sb, \
             tc.tile_pool(name="p3half", bufs=16) as p3half, \
             tc.tile_pool(name="p3ps", bufs=3, space="PSUM") as p3ps:
            halves = {}
            for tt in range(4):
                t0 = tt * 128
                for fc in range(4):
                    f0 = fc * 512
                    ps = p3ps.tile([128, 512], F32, tag="o_ps")
                    nc.tensor.matmul(ps[:], lhsT=onesb[:], rhs=bo[:, f0:f0 + 512],
                                     start=True, stop=False)
                    for dt in range(8):
                        nc.tensor.matmul(ps[:], lhsT=ctxg[0][:, dt, t0:t0 + 128],
                                         rhs=wo[:, dt, f0:f0 + 512],
                                         start=False, stop=(dt == 7))
                    half = p3half.tile([128, 512], F32, tag="half")
                    nc.scalar.copy(half[:], ps[:])
                    halves[(tt, fc)] = half
            for tt in range(4):
                t0 = tt * 128
                for fc in range(4):
                    f0 = fc * 512
                    ps = p3ps.tile([128, 512], F32, tag="o_ps")
                    for dt in range(8, N_DT):
                        nc.tensor.matmul(ps[:], lhsT=ctxg[1][:, dt - 8, t0:t0 + 128],
                                         rhs=wo[:, dt, f0:f0 + 512],
                                         start=(dt == 8), stop=(dt == N_DT - 1))
                    osb = p3sb.tile([128, 512], F32, tag="osb")
                    nc.vector.tensor_add(osb[:], ps[:], halves[(tt, fc)][:])
                    nc.sync.dma_start(out=out_ext[t0:t0 + 128, f0:f0 + 512], in_=osb[:])


def _prep(x, freqs, W_qkv, b_qkv, W_o, b_o):
    """Host-side sharding/layout. Returns in_maps for the 8 cores."""
    perm = np.concatenate([np.arange(0, 128, 2), np.arange(1, 128, 2)])  # even dims first

    x_t = np.ascontiguousarray(x.transpose(2, 1, 0).reshape(D_MODEL, T)).astype(BF16_NP)

    cos = np.cos(freqs).astype(np.float32)       # [SEQ, 64]
    sin = np.sin(freqs).astype(np.float32)
    cosT = np.empty((2, 128, SEQ), np.float32)
    sinT = np.empty((2, 128, SEQ), np.float32)
    cosT[0, 0:64] = cos.T
    cosT[0, 64:128] = cos.T
    sinT[0, 0:64] = sin.T                        # bottom-half output uses +sin
    sinT[0, 64:128] = -sin.T                     # top-half output uses -sin
    scale = 1.0 / np.sqrt(np.float32(D_HEAD))
    cosT[1] = cosT[0] * scale
    sinT[1] = sinT[0] * scale
    cosT = cosT.astype(BF16_NP)
    sinT = sinT.astype(BF16_NP)

    m = np.empty((4, 128, 512), np.float32)
    p_idx = np.arange(128)[:, None]
    f_idx = np.arange(512)[None, :]
    for d in range(4):
        m[d] = (f_idx >= p_idx + 128 * d).astype(np.float32)
    masks = m.astype(BF16_NP)

    # W_o rows reordered: even global heads then odd (A2A #0 carries local head 0
    # of every core = even global heads)
    wo_order = np.concatenate([np.arange(N_HEADS)[::2], np.arange(N_HEADS)[1::2]])
    wo_t = np.ascontiguousarray(
        W_o.T.reshape(N_HEADS, D_HEAD, D_MODEL)[wo_order].reshape(D_MODEL, D_MODEL)
    ).astype(BF16_NP)
    bo = np.ascontiguousarray(b_o[None, :]).astype(BF16_NP)

    in_maps = []
    for c in range(N_CORES):
        rows = slice(256 * c, 256 * (c + 1))
        wq = W_qkv[0 * D_MODEL:1 * D_MODEL][rows].reshape(2, 128, D_MODEL)[:, perm]
        wk = W_qkv[1 * D_MODEL:2 * D_MODEL][rows].reshape(2, 128, D_MODEL)[:, perm]
        wv = W_qkv[2 * D_MODEL:3 * D_MODEL][rows]
        bq = b_qkv[0 * D_MODEL:1 * D_MODEL][rows].reshape(2, 128)[:, perm]
        bk = b_qkv[1 * D_MODEL:2 * D_MODEL][rows].reshape(2, 128)[:, perm]
        bv = b_qkv[2 * D_MODEL:3 * D_MODEL][rows]
        wqk = np.ascontiguousarray(
            np.concatenate([wq.reshape(256, D_MODEL), wk.reshape(256, D_MODEL)]).T
        ).astype(BF16_NP)
        wv_t = np.ascontiguousarray(wv.T).astype(BF16_NP)
        in_maps.append({
            "xt": x_t, "wqk": wqk,
            "bqk": np.ascontiguousarray(np.concatenate([bq, bk])).astype(np.float32),
            "wv": wv_t, "bv": np.ascontiguousarray(bv.reshape(2, 128)).astype(np.float32),
            "ident": np.eye(128, dtype=BF16_NP),
            "cosT": cosT, "sinT": sinT, "masks": masks,
            "wo": wo_t, "bo": bo,
        })
    return in_maps


def kernel(x, freqs, W_qkv, b_qkv, W_o, b_o, _trace=False, _tmpdir=None):
    from concourse.bass_utils import run_bass_kernel_spmd

    in_maps = _prep(np.asarray(x, np.float32), np.asarray(freqs, np.float32),
                    np.asarray(W_qkv, np.float32), np.asarray(b_qkv, np.float32),
                    np.asarray(W_o, np.float32), np.asarray(b_o, np.float32))
    if "nc" not in _cached:
        _cached["nc"] = build()
    res = run_bass_kernel_spmd(_cached["nc"], in_maps, core_ids=list(range(N_CORES)),
                               trace=_trace, tmpdir=_tmpdir)
    _cached["last_result"] = res
    full = np.concatenate([res.results[c]["out"] for c in range(N_CORES)], axis=0)
    # batch-major [T, D] -> (SEQ, BATCH, D)
    return np.ascontiguousarray(
        full.reshape(BATCH, SEQ, D_MODEL).transpose(1, 0, 2)).astype(np.float32)


# revision 17
# speedup vs baseline: 1.0091x; 1.0091x over previous
"""Causal multi-head attention (S=2048, B=2, D=2048, H=16, dh=128) on 8 TRN2
NeuronCores.

Sharding: tensor-parallel by heads. Core c owns heads {2c, 2c+1}: it projects
q/k/v for those heads from the full x, applies RoPE, runs causal attention,
then an AllToAll re-shards the context from head-split to token-split and each
core computes its 512-token slice of the output projection. Host-side work is
layout only: transposes, per-head even/odd permutation of W_q/W_k rows (so the
RoPE pair-swap becomes a 64-partition block swap), cos/sin tables from freqs,
causal mask tiles, and the final concat of per-core token slices.

Compute is bf16 (f32 PSUM accumulation); softmax runs without max-subtraction
(scores are O(1) by construction: x ~ N(0,1), W ~ N(0, 1/D)).
"""
import numpy as np
import ml_dtypes

import concourse.bass as bass
import concourse.bacc as bacc
import concourse.mybir as mybir
import concourse.tile as tile

N_CORES = 8
D_MODEL = 2048
N_HEADS = 16
D_HEAD = 128
SEQ = 2048
BATCH = 2
T = SEQ * BATCH          # 4096 tokens, batch-major: t = b*SEQ + s
H_PER_CORE = 2           # heads per core
E_QK = 512               # q+k features per core (2 heads x 128 x 2)
E_V = 256                # v features per core
N_DT = D_MODEL // 128    # 16 d-tiles
N_CHUNK = T // 512       # 8 token chunks of 512
SQ_CHUNKS = SEQ // 512   # 4 query chunks per (b,h) pair
SK_TILES = SEQ // 128    # 16 key tiles per (b,h) pair
T_SLICE = T // N_CORES   # 512 tokens per core in the output phase

F32 = mybir.dt.float32
BF16 = mybir.dt.bfloat16
BF16_NP = ml_dtypes.bfloat16

_cached = {}


def build():
    nc = bacc.Bacc("TRN2", target_bir_lowering=False, debug=False, num_devices=N_CORES)

    xt_ext = nc.declare_dram_parameter("xt", [D_MODEL, T], BF16, isOutput=False)
    wqk_ext = nc.declare_dram_parameter("wqk", [D_MODEL, E_QK], BF16, isOutput=False)
    bqk_ext = nc.declare_dram_parameter("bqk", [4, 128], F32, isOutput=False)
    wv_ext = nc.declare_dram_parameter("wv", [D_MODEL, E_V], BF16, isOutput=False)
    bv_ext = nc.declare_dram_parameter("bv", [2, 128], F32, isOutput=False)
    id_ext = nc.declare_dram_parameter("ident", [128, 128], BF16, isOutput=False)
    cos_ext = nc.declare_dram_parameter("cosT", [2, 128, SEQ], BF16, isOutput=False)
    sin_ext = nc.declare_dram_parameter("sinT", [2, 128, SEQ], BF16, isOutput=False)
    mask_ext = nc.declare_dram_parameter("masks", [4, 128, 512], BF16, isOutput=False)
    wo_ext = nc.declare_dram_parameter("wo", [D_MODEL, D_MODEL], BF16, isOutput=False)
    bo_ext = nc.declare_dram_parameter("bo", [1, D_MODEL], BF16, isOutput=False)
    out_ext = nc.declare_dram_parameter("out", [T_SLICE, D_MODEL], F32, isOutput=True)

    with tile.TileContext(nc) as tc:
        _body(nc, tc, xt_ext, wqk_ext, bqk_ext, wv_ext, bv_ext, id_ext, cos_ext, sin_ext,
              mask_ext, wo_ext, bo_ext, out_ext)
    nc.compile()
    return nc


def _body(nc, tc, xt_ext, wqk_ext, bqk_ext, wv_ext, bv_ext, id_ext, cos_ext, sin_ext,
          mask_ext, wo_ext, bo_ext, out_ext):
    EXP = mybir.ActivationFunctionType.Exp
    IDENT = mybir.ActivationFunctionType.Identity

    with tc.tile_pool(name="res", bufs=1) as res, \
         tc.tile_pool(name="dram", bufs=1, space="DRAM") as dram:
        # ---- resident tiles -------------------------------------------------
        wqk = res.tile([128, N_DT, E_QK], BF16, tag="wqk")
        wv = res.tile([128, N_DT, E_V], BF16, tag="wv")
        masks = res.tile([128, 4, 512], BF16, tag="masks")
        bqk = res.tile([128, 4], F32, tag="bqk")
        bv = res.tile([128, 2], F32, tag="bv")
        ident = res.tile([128, 128], BF16, tag="ident")
        bo = res.tile([1, D_MODEL], BF16, tag="bo")
        ones128 = res.tile([128, 1], BF16, tag="ones128")
        ones1 = res.tile([1, 128], F32, tag="ones1")
        onesb = res.tile([1, 128], BF16, tag="onesb")

        q = res.tile([128, 4, SEQ], BF16, tag="q")     # [dh, pair, s]
        k = res.tile([128, 4, SEQ], BF16, tag="k")
        v = res.tile([128, 4, SK_TILES, D_HEAD], BF16, tag="v")  # [sk_in_tile, pair, sk_tile, dh]

        a2a_in = [dram.tile([8, 128, 512], BF16, name=f"a2a_in{i}", tag=f"a2a_in{i}") for i in range(2)]
        a2a_out = [dram.tile([8, 128, 512], BF16, name=f"a2a_out{i}", tag=f"a2a_out{i}") for i in range(2)]

        nc.scalar.dma_start(out=wqk[:], in_=wqk_ext[:, :].rearrange("(n p) e -> p n e", p=128))
        nc.scalar.dma_start(out=wv[:], in_=wv_ext[:, :].rearrange("(n p) e -> p n e", p=128))
        nc.scalar.dma_start(out=masks[:], in_=mask_ext[:, :, :].rearrange("m p f -> p m f"))
        nc.scalar.dma_start(out=bqk[:], in_=bqk_ext[:, :].rearrange("e p -> p e"))
        nc.scalar.dma_start(out=bv[:], in_=bv_ext[:, :].rearrange("e p -> p e"))
        nc.scalar.dma_start(out=ident[:], in_=id_ext[:, :])
        nc.scalar.dma_start(out=bo[:], in_=bo_ext[:, :])
        nc.gpsimd.memset(ones128[:], 1.0)
        nc.gpsimd.memset(ones1[:], 1.0)
        nc.gpsimd.memset(onesb[:], 1.0)

        # ---- phase 1: QKV projection + RoPE (1024-wide token chunks) -------
        with tc.tile_pool(name="p1sb", bufs=2) as p1sb, \
             tc.tile_pool(name="p1tab", bufs=1) as p1tab, \
             tc.tile_pool(name="p1tmp", bufs=4) as p1tmp, \
             tc.tile_pool(name="p1ps", bufs=3, space="PSUM") as p1ps, \
             tc.tile_pool(name="p1tp", bufs=2, space="PSUM") as p1tp:
            cosT = p1tab.tile([128, 2, SEQ], BF16, tag="cosT")   # [part, qk, s]
            sinT = p1tab.tile([128, 2, SEQ], BF16, tag="sinT")
            nc.scalar.dma_start(out=cosT[:], in_=cos_ext[:, :, :].rearrange("i p s -> p i s"))
            nc.scalar.dma_start(out=sinT[:], in_=sin_ext[:, :, :].rearrange("i p s -> p i s"))
            for ch in range(4):
                b, half = divmod(ch, 2)
                s0 = half * 1024
                xc = p1sb.tile([128, N_DT, 1024], BF16, tag="xc")
                for qd in range(4):
                    nc.sync.dma_start(
                        out=xc[:, qd * 4:(qd + 1) * 4, :],
                        in_=xt_ext[qd * 512:(qd + 1) * 512, ch * 1024:(ch + 1) * 1024]
                        .rearrange("(n p) t -> p n t", p=128))

                # q/k/v^T: psum[e,128 x t,1024] accumulated over 16 d-tiles
                # et 0,1: q h0,h1; 2,3: k h0,h1; 4,5: v h0,h1 (transposed after)
                for et in range(6):
                    ps = p1ps.tile([128, 1024], F32, tag="qk_ps")
                    for u in range(2):
                        for dt in range(N_DT):
                            if et < 4:
                                lhsT = wqk[:, dt, et * 128:(et + 1) * 128]
                            else:
                                lhsT = wv[:, dt, (et - 4) * 128:(et - 3) * 128]
                            nc.tensor.matmul(ps[:, u * 512:(u + 1) * 512], lhsT=lhsT,
                                             rhs=xc[:, dt, u * 512:(u + 1) * 512],
                                             start=(dt == 0), stop=(dt == N_DT - 1))
                    raw = p1tmp.tile([128, 1024], BF16, tag="qkraw")
                    if et < 4:
                        nc.scalar.activation(raw[:], ps[:], IDENT, bias=bqk[:, et:et + 1])
                        # RoPE: dest = raw*cos + blockswap(raw)*sin (sin pre-signed/swapped)
                        qk_i = 0 if et < 2 else 1        # q tables / k tables (scaled)
                        pair = (et % 2) * 2 + b
                        dest = (q if et < 2 else k)[:, pair, s0:s0 + 1024]
                        cs = cosT[:, qk_i, s0:s0 + 1024]
                        sn = sinT[:, qk_i, s0:s0 + 1024]
                        tmp = p1tmp.tile([128, 1024], BF16, tag="ropetmp")
                        nc.vector.tensor_mul(dest, raw[:], cs)
                        nc.vector.tensor_mul(tmp[0:64, :], raw[64:128, :], sn[64:128, :])
                        nc.vector.tensor_mul(tmp[64:128, :], raw[0:64, :], sn[0:64, :])
                        nc.vector.tensor_add(dest, dest, tmp[:])
                    else:
                        hv = et - 4
                        nc.scalar.activation(raw[:], ps[:], IDENT, bias=bv[:, hv:hv + 1])
                        pair = hv * 2 + b
                        for blk in range(8):
                            tp = p1tp.tile([128, 128], BF16, tag="tp")
                            nc.tensor.transpose(tp[:], raw[:, blk * 128:(blk + 1) * 128],
                                                ident[:])
                            nc.vector.tensor_copy(v[:, pair, half * 8 + blk, :], tp[:])

        # ---- phases 2+3 ----------------------------------------------------
        with tc.tile_pool(name="late", bufs=1) as late:
            wo = late.tile([128, N_DT, D_MODEL], BF16, tag="wo")
            ctxg = [late.tile([128, 8, 512], BF16, name=f"ctxg{i}", tag=f"ctxg{i}") for i in range(2)]
            nc.scalar.dma_start(out=wo[:], in_=wo_ext[:, :].rearrange("(n p) e -> p n e", p=128))
            _phase23(nc, tc, q, k, v, masks, ones128, ones1, onesb, bo, wo, ctxg,
                     a2a_in, a2a_out, out_ext)


def _phase23(nc, tc, q, k, v, masks, ones128, ones1, onesb, bo, wo, ctxg,
             a2a_in, a2a_out, out_ext):
        EXP = mybir.ActivationFunctionType.Exp
        # ---- phase 2: causal attention, per (head, batch) pair -------------
        with tc.tile_pool(name="p2exp", bufs=6) as p2exp, \
             tc.tile_pool(name="p2tmp", bufs=3) as p2tmp, \
             tc.tile_pool(name="p2dt", bufs=6) as p2dt, \
             tc.tile_pool(name="p2ps", bufs=2, space="PSUM") as p2ps, \
             tc.tile_pool(name="p2ctx", bufs=2, space="PSUM") as p2ctx, \
             tc.tile_pool(name="p2dn", bufs=2, space="PSUM") as p2dn:
            for h in range(2):                   # local head; A2A #h after its 2 pairs
                for j in range(SQ_CHUNKS):
                    sq0 = j * 512
                    n_sk = 4 * (j + 1)
                    ctx_ps = {}
                    dn_ps = {}
                    for b in range(BATCH):
                        ctx_ps[b] = p2ctx.tile([128, 512], F32, name=f"ctx_ps{b}", tag="ctx_ps")
                        dn_ps[b] = p2dn.tile([1, 512], F32, name=f"dn_ps{b}", tag="dnbc")
                    prev_ex = {}
                    for g in range(n_sk // 2):       # groups of 2 sk-tiles
                        for b in range(BATCH):       # interleave the two batches
                            pair = h * 2 + b
                            sc_ps = p2ps.tile([128, 1024], F32, name=f"sc_ps{b}", tag="sc_ps")
                            for u in range(2):
                                i = 2 * g + u
                                nc.tensor.matmul(sc_ps[:, u * 512:(u + 1) * 512],
                                                 lhsT=k[:, pair, i * 128:(i + 1) * 128],
                                                 rhs=q[:, pair, sq0:sq0 + 512],
                                                 start=True, stop=True)
                            ex = p2exp.tile([128, 1024], BF16, name=f"ex{b}", tag="ex")
                            nc.scalar.activation(ex[:], sc_ps[:], EXP)
                            if 2 * g >= 4 * j:           # diagonal group: causal mask
                                m = 2 * g - 4 * j        # 0 or 2
                                nc.vector.tensor_mul(ex[:], ex[:], masks[:, m:m + 2, :]
                                                     .rearrange("p m f -> p (m f)"))
                            for u in range(2):
                                i = 2 * g + u
                                nc.tensor.matmul(ctx_ps[b][:], lhsT=v[:, pair, i, :],
                                                 rhs=ex[:, u * 512:(u + 1) * 512],
                                                 start=(i == 0), stop=(i == n_sk - 1))
                            # denominator: tree-sum 4 sk tiles in bf16, then one
                            # ones-matmul per quad into the f32 psum accumulator
                            t1 = p2dt.tile([128, 512], BF16, name=f"t1_{b}", tag="dtree")
                            nc.vector.tensor_add(t1[:], ex[:, 0:512], ex[:, 512:1024])
                            if g % 2 == 0:
                                prev_ex[b] = t1
                            else:
                                t3 = p2dt.tile([128, 512], BF16, name=f"t3_{b}", tag="dtree")
                                nc.vector.tensor_add(t3[:], t1[:], prev_ex[b][:])
                                nc.tensor.matmul(dn_ps[b][:], lhsT=ones128[:], rhs=t3[:],
                                                 start=(g == 1), stop=(g == n_sk // 2 - 1))
                    for b in range(BATCH):
                        # normalize: ctx * (1/denom) broadcast across partitions
                        recip = p2tmp.tile([1, 512], F32, name=f"recip{b}", tag="recip")
                        nc.vector.reciprocal_approx_fast(out=recip[:], in_=dn_ps[b][:])
                        bc_ps = p2dn.tile([128, 512], F32, name=f"bc_ps{b}", tag="dnbc")
                        nc.tensor.matmul(bc_ps[:], lhsT=ones1[:], rhs=recip[:],
                                         start=True, stop=True)
                        bc = p2tmp.tile([128, 512], F32, name=f"bc{b}", tag="bc")
                        nc.scalar.copy(bc[:], bc_ps[:])
                        ctx_sb = p2tmp.tile([128, 512], BF16, name=f"ctx_sb{b}", tag="ctx_sb")
                        nc.vector.tensor_mul(ctx_sb[:], ctx_ps[b][:], bc[:])
                        nc.sync.dma_start(out=a2a_in[h][4 * b + j, :, :], in_=ctx_sb[:])
                nc.gpsimd.collective_compute(
                    "AllToAll", mybir.AluOpType.bypass,
                    replica_groups=[list(range(N_CORES))],
                    ins=[a2a_in[h][:, :, :].opt()],
                    outs=[a2a_out[h][:, :, :].opt()])
                nc.sync.dma_start(out=ctxg[h][:],
                                  in_=a2a_out[h][:, :, :].rearrange("j p t -> p j t"))

        # ---- phase 3: output projection on this core's 512-token slice -----
        # split over the two A2A halves: even-head dims right after A2A#0
        # (overlaps A2A#1), odd-head dims after A2A#1.
        with tc.tile_pool(name="p3sb", bufs=3) as p3sb, \
             tc.tile_pool(name="p3half", bufs=16) as p3half, \
             tc.tile_pool(name="p3ps", bufs=3, space="PSUM") as p3ps:
            halves = {}
            for tt in range(4):
                t0 = tt * 128
                for fc in range(4):
                    f0 = fc * 512
                    ps = p3ps.tile([128, 512], F32, tag="o_ps")
                    nc.tensor.matmul(ps[:], lhsT=onesb[:], rhs=bo[:, f0:f0 + 512],
                                     start=True, stop=False)
                    for dt in range(8):
                        nc.tensor.matmul(ps[:], lhsT=ctxg[0][:, dt, t0:t0 + 128],
                                         rhs=wo[:, dt, f0:f0 + 512],
                                         start=False, stop=(dt == 7))
                    half = p3half.tile([128, 512], F32, tag="half")
                    nc.scalar.copy(half[:], ps[:])
                    halves[(tt, fc)] = half
            for tt in range(4):
                t0 = tt * 128
                for fc in range(4):
                    f0 = fc * 512
                    ps = p3ps.tile([128, 512], F32, tag="o_ps")
                    for dt in range(8, N_DT):
                        nc.tensor.matmul(ps[:], lhsT=ctxg[1][:, dt - 8, t0:t0 + 128],
                                         rhs=wo[:, dt, f0:f0 + 512],
                                         start=(dt == 8), stop=(dt == N_DT - 1))
                    osb = p3sb.tile([128, 512], F32, tag="osb")
                    nc.vector.tensor_add(osb[:], ps[:], halves[(tt, fc)][:])
                    nc.sync.dma_start(out=out_ext[t0:t0 + 128, f0:f0 + 512], in_=osb[:])


def _prep(x, freqs, W_qkv, b_qkv, W_o, b_o):
    """Host-side sharding/layout. Returns in_maps for the 8 cores."""
    perm = np.concatenate([np.arange(0, 128, 2), np.arange(1, 128, 2)])  # even dims first

    x_t = np.ascontiguousarray(x.transpose(2, 1, 0).reshape(D_MODEL, T)).astype(BF16_NP)

    cos = np.cos(freqs).astype(np.float32)       # [SEQ, 64]
    sin = np.sin(freqs).astype(np.float32)
    cosT = np.empty((2, 128, SEQ), np.float32)
    sinT = np.empty((2, 128, SEQ), np.float32)
    cosT[0, 0:64] = cos.T
    cosT[0, 64:128] = cos.T
    sinT[0, 0:64] = sin.T                        # bottom-half output uses +sin
    sinT[0, 64:128] = -sin.T                     # top-half output uses -sin
    scale = 1.0 / np.sqrt(np.float32(D_HEAD))
    cosT[1] = cosT[0] * scale
    sinT[1] = sinT[0] * scale
    cosT = cosT.astype(BF16_NP)
    sinT = sinT.astype(BF16_NP)

    m = np.empty((4, 128, 512), np.float32)
    p_idx = np.arange(128)[:, None]
    f_idx = np.arange(512)[None, :]
    for d in range(4):
        m[d] = (f_idx >= p_idx + 128 * d).astype(np.float32)
    masks = m.astype(BF16_NP)

    # W_o rows reordered: even global heads then odd (A2A #0 carries local head 0
    # of every core = even global heads)
    wo_order = np.concatenate([np.arange(N_HEADS)[::2], np.arange(N_HEADS)[1::2]])
    wo_t = np.ascontiguousarray(
        W_o.T.reshape(N_HEADS, D_HEAD, D_MODEL)[wo_order].reshape(D_MODEL, D_MODEL)
    ).astype(BF16_NP)
    bo = np.ascontiguousarray(b_o[None, :]).astype(BF16_NP)

    in_maps = []
    for c in range(N_CORES):
        rows = slice(256 * c, 256 * (c + 1))
        wq = W_qkv[0 * D_MODEL:1 * D_MODEL][rows].reshape(2, 128, D_MODEL)[:, perm]
        wk = W_qkv[1 * D_MODEL:2 * D_MODEL][rows].reshape(2, 128, D_MODEL)[:, perm]
        wv = W_qkv[2 * D_MODEL:3 * D_MODEL][rows]
        bq = b_qkv[0 * D_MODEL:1 * D_MODEL][rows].reshape(2, 128)[:, perm]
        bk = b_qkv[1 * D_MODEL:2 * D_MODEL][rows].reshape(2, 128)[:, perm]
        bv = b_qkv[2 * D_MODEL:3 * D_MODEL][rows]
        wqk = np.ascontiguousarray(
            np.concatenate([wq.reshape(256, D_MODEL), wk.reshape(256, D_MODEL)]).T
        ).astype(BF16_NP)
        wv_t = np.ascontiguousarray(wv.T).astype(BF16_NP)
        in_maps.append({
            "xt": x_t, "wqk": wqk,
            "bqk": np.ascontiguousarray(np.concatenate([bq, bk])).astype(np.float32),
            "wv": wv_t, "bv": np.ascontiguousarray(bv.reshape(2, 128)).astype(np.float32),
            "ident": np.eye(128, dtype=BF16_NP),
            "cosT": cosT, "sinT": sinT, "masks": masks,
            "wo": wo_t, "bo": bo,
        })
    return in_maps


def kernel(x, freqs, W_qkv, b_qkv, W_o, b_o, _trace=False, _tmpdir=None):
    from concourse.bass_utils import run_bass_kernel_spmd

    in_maps = _prep(np.asarray(x, np.float32), np.asarray(freqs, np.float32),
                    np.asarray(W_qkv, np.float32), np.asarray(b_qkv, np.float32),
                    np.asarray(W_o, np.float32), np.asarray(b_o, np.float32))
    if "nc" not in _cached:
        _cached["nc"] = build()
    res = run_bass_kernel_spmd(_cached["nc"], in_maps, core_ids=list(range(N_CORES)),
                               trace=_trace, tmpdir=_tmpdir)
    _cached["last_result"] = res
    full = np.concatenate([res.results[c]["out"] for c in range(N_CORES)], axis=0)
    # batch-major [T, D] -> (SEQ, BATCH, D)
    return np.ascontiguousarray(
        full.reshape(BATCH, SEQ, D_MODEL).transpose(1, 0, 2)).astype(np.float32)


# revision 18
# speedup vs baseline: 1.0831x; 1.0734x over previous
"""Causal multi-head attention (S=2048, B=2, D=2048, H=16, dh=128) on 8 TRN2
NeuronCores.

Sharding: tensor-parallel by heads. Core c owns heads {2c, 2c+1}: it projects
q/k/v for those heads from the full x, applies RoPE, runs causal attention,
then an AllToAll re-shards the context from head-split to token-split and each
core computes its 512-token slice of the output projection. Host-side work is
layout only: transposes, per-head even/odd permutation of W_q/W_k rows (so the
RoPE pair-swap becomes a 64-partition block swap), cos/sin tables from freqs,
causal mask tiles, and the final concat of per-core token slices.

Compute is bf16 (f32 PSUM accumulation); softmax runs without max-subtraction
(scores are O(1) by construction: x ~ N(0,1), W ~ N(0, 1/D)).
"""
import numpy as np
import ml_dtypes

import concourse.bass as bass
import concourse.bacc as bacc
import concourse.mybir as mybir
import concourse.tile as tile

N_CORES = 8
D_MODEL = 2048
N_HEADS = 16
D_HEAD = 128
SEQ = 2048
BATCH = 2
T = SEQ * BATCH          # 4096 tokens, batch-major: t = b*SEQ + s
H_PER_CORE = 2           # heads per core
E_QK = 512               # q+k features per core (2 heads x 128 x 2)
E_V = 256                # v features per core
N_DT = D_MODEL // 128    # 16 d-tiles
N_CHUNK = T // 512       # 8 token chunks of 512
SQ_CHUNKS = SEQ // 512   # 4 query chunks per (b,h) pair
SK_TILES = SEQ // 128    # 16 key tiles per (b,h) pair
T_SLICE = T // N_CORES   # 512 tokens per core in the output phase

F32 = mybir.dt.float32
BF16 = mybir.dt.bfloat16
BF16_NP = ml_dtypes.bfloat16

_cached = {}


def build():
    nc = bacc.Bacc("TRN2", target_bir_lowering=False, debug=False, num_devices=N_CORES)

    xt_ext = nc.declare_dram_parameter("xt", [D_MODEL, T], BF16, isOutput=False)
    wqk_ext = nc.declare_dram_parameter("wqk", [D_MODEL, E_QK], BF16, isOutput=False)
    bqk_ext = nc.declare_dram_parameter("bqk", [4, 128], F32, isOutput=False)
    wv_ext = nc.declare_dram_parameter("wv", [D_MODEL, E_V], BF16, isOutput=False)
    bv_ext = nc.declare_dram_parameter("bv", [2, 128], F32, isOutput=False)
    id_ext = nc.declare_dram_parameter("ident", [128, 128], BF16, isOutput=False)
    cos_ext = nc.declare_dram_parameter("cosT", [2, 128, SEQ], BF16, isOutput=False)
    sin_ext = nc.declare_dram_parameter("sinT", [2, 128, SEQ], BF16, isOutput=False)
    mask_ext = nc.declare_dram_parameter("masks", [4, 128, 512], BF16, isOutput=False)
    wo_ext = nc.declare_dram_parameter("wo", [D_MODEL, D_MODEL], BF16, isOutput=False)
    bo_ext = nc.declare_dram_parameter("bo", [1, D_MODEL], BF16, isOutput=False)
    out_ext = nc.declare_dram_parameter("out", [T_SLICE, D_MODEL], F32, isOutput=True)

    with tile.TileContext(nc) as tc:
        _body(nc, tc, xt_ext, wqk_ext, bqk_ext, wv_ext, bv_ext, id_ext, cos_ext, sin_ext,
              mask_ext, wo_ext, bo_ext, out_ext)
    nc.compile()
    return nc


def _body(nc, tc, xt_ext, wqk_ext, bqk_ext, wv_ext, bv_ext, id_ext, cos_ext, sin_ext,
          mask_ext, wo_ext, bo_ext, out_ext):
    EXP = mybir.ActivationFunctionType.Exp
    IDENT = mybir.ActivationFunctionType.Identity

    with tc.tile_pool(name="res", bufs=1) as res, \
         tc.tile_pool(name="dram", bufs=1, space="DRAM") as dram:
        # ---- resident tiles -------------------------------------------------
        wqk = res.tile([128, N_DT, E_QK], BF16, tag="wqk")
        wv = res.tile([128, N_DT, E_V], BF16, tag="wv")
        masks = res.tile([128, 4, 512], BF16, tag="masks")
        bqk = res.tile([128, 4], F32, tag="bqk")
        bv = res.tile([128, 2], F32, tag="bv")
        ident = res.tile([128, 128], BF16, tag="ident")
        bo = res.tile([1, D_MODEL], BF16, tag="bo")
        ones128 = res.tile([128, 1], BF16, tag="ones128")
        ones1 = res.tile([1, 128], F32, tag="ones1")
        onesb = res.tile([1, 128], BF16, tag="onesb")

        q = res.tile([128, 4, SEQ], BF16, tag="q")     # [dh, pair, s]
        k = res.tile([128, 4, SEQ], BF16, tag="k")
        v = res.tile([128, 4, SK_TILES, D_HEAD], BF16, tag="v")  # [sk_in_tile, pair, sk_tile, dh]

        a2a_in = [dram.tile([8, 128, 512], BF16, name=f"a2a_in{i}", tag=f"a2a_in{i}") for i in range(2)]
        a2a_out = [dram.tile([8, 128, 512], BF16, name=f"a2a_out{i}", tag=f"a2a_out{i}") for i in range(2)]

        for qd in range(4):
            nc.scalar.dma_start(out=wqk[:, qd * 4:(qd + 1) * 4, :],
                                in_=wqk_ext[qd * 512:(qd + 1) * 512, :]
                                .rearrange("(n p) e -> p n e", p=128))
        nc.scalar.dma_start(out=wv[:], in_=wv_ext[:, :].rearrange("(n p) e -> p n e", p=128))
        nc.scalar.dma_start(out=masks[:], in_=mask_ext[:, :, :].rearrange("m p f -> p m f"))
        nc.scalar.dma_start(out=bqk[:], in_=bqk_ext[:, :].rearrange("e p -> p e"))
        nc.scalar.dma_start(out=bv[:], in_=bv_ext[:, :].rearrange("e p -> p e"))
        nc.scalar.dma_start(out=ident[:], in_=id_ext[:, :])
        nc.scalar.dma_start(out=bo[:], in_=bo_ext[:, :])
        nc.gpsimd.memset(ones128[:], 1.0)
        nc.gpsimd.memset(ones1[:], 1.0)
        nc.gpsimd.memset(onesb[:], 1.0)

        # ---- phase 1: QKV projection + RoPE (1024-wide token chunks) -------
        with tc.tile_pool(name="p1sb", bufs=2) as p1sb, \
             tc.tile_pool(name="p1tab", bufs=1) as p1tab, \
             tc.tile_pool(name="p1tmp", bufs=4) as p1tmp, \
             tc.tile_pool(name="p1ps", bufs=3, space="PSUM") as p1ps, \
             tc.tile_pool(name="p1tp", bufs=2, space="PSUM") as p1tp:
            cosT = p1tab.tile([128, 2, SEQ], BF16, tag="cosT")   # [part, qk, s]
            sinT = p1tab.tile([128, 2, SEQ], BF16, tag="sinT")
            nc.scalar.dma_start(out=cosT[:], in_=cos_ext[:, :, :].rearrange("i p s -> p i s"))
            nc.scalar.dma_start(out=sinT[:], in_=sin_ext[:, :, :].rearrange("i p s -> p i s"))
            for ch in range(4):
                b, half = divmod(ch, 2)
                s0 = half * 1024
                xc = p1sb.tile([128, N_DT, 1024], BF16, tag="xc")
                for qd in range(4):
                    nc.sync.dma_start(
                        out=xc[:, qd * 4:(qd + 1) * 4, :],
                        in_=xt_ext[qd * 512:(qd + 1) * 512, ch * 1024:(ch + 1) * 1024]
                        .rearrange("(n p) t -> p n t", p=128))

                # q/k/v^T: psum[e,128 x t,1024] accumulated over 16 d-tiles
                # et 0,1: q h0,h1; 2,3: k h0,h1; 4,5: v h0,h1 (transposed after)
                for et in range(6):
                    ps = p1ps.tile([128, 1024], F32, tag="qk_ps")
                    for u in range(2):
                        for dt in range(N_DT):
                            if et < 4:
                                lhsT = wqk[:, dt, et * 128:(et + 1) * 128]
                            else:
                                lhsT = wv[:, dt, (et - 4) * 128:(et - 3) * 128]
                            nc.tensor.matmul(ps[:, u * 512:(u + 1) * 512], lhsT=lhsT,
                                             rhs=xc[:, dt, u * 512:(u + 1) * 512],
                                             start=(dt == 0), stop=(dt == N_DT - 1))
                    raw = p1tmp.tile([128, 1024], BF16, tag="qkraw")
                    if et < 4:
                        nc.scalar.activation(raw[:], ps[:], IDENT, bias=bqk[:, et:et + 1])
                        # RoPE: dest = raw*cos + blockswap(raw)*sin (sin pre-signed/swapped)
                        qk_i = 0 if et < 2 else 1        # q tables / k tables (scaled)
                        pair = (et % 2) * 2 + b
                        dest = (q if et < 2 else k)[:, pair, s0:s0 + 1024]
                        cs = cosT[:, qk_i, s0:s0 + 1024]
                        sn = sinT[:, qk_i, s0:s0 + 1024]
                        tmp = p1tmp.tile([128, 1024], BF16, tag="ropetmp")
                        nc.vector.tensor_mul(dest, raw[:], cs)
                        nc.vector.tensor_mul(tmp[0:64, :], raw[64:128, :], sn[64:128, :])
                        nc.vector.tensor_mul(tmp[64:128, :], raw[0:64, :], sn[0:64, :])
                        nc.vector.tensor_add(dest, dest, tmp[:])
                    else:
                        hv = et - 4
                        nc.scalar.activation(raw[:], ps[:], IDENT, bias=bv[:, hv:hv + 1])
                        pair = hv * 2 + b
                        for blk in range(8):
                            tp = p1tp.tile([128, 128], BF16, tag="tp")
                            nc.tensor.transpose(tp[:], raw[:, blk * 128:(blk + 1) * 128],
                                                ident[:])
                            nc.vector.tensor_copy(v[:, pair, half * 8 + blk, :], tp[:])

        # ---- phases 2+3 ----------------------------------------------------
        with tc.tile_pool(name="late", bufs=1) as late:
            wo = late.tile([128, N_DT, D_MODEL], BF16, tag="wo")
            ctxg = [late.tile([128, 8, 512], BF16, name=f"ctxg{i}", tag=f"ctxg{i}") for i in range(2)]
            nc.scalar.dma_start(out=wo[:], in_=wo_ext[:, :].rearrange("(n p) e -> p n e", p=128))
            _phase23(nc, tc, q, k, v, masks, ones128, ones1, onesb, bo, wo, ctxg,
                     a2a_in, a2a_out, out_ext)


def _phase23(nc, tc, q, k, v, masks, ones128, ones1, onesb, bo, wo, ctxg,
             a2a_in, a2a_out, out_ext):
        EXP = mybir.ActivationFunctionType.Exp
        # ---- phase 2: causal attention, per (head, batch) pair -------------
        with tc.tile_pool(name="p2exp", bufs=6) as p2exp, \
             tc.tile_pool(name="p2tmp", bufs=3) as p2tmp, \
             tc.tile_pool(name="p2dt", bufs=6) as p2dt, \
             tc.tile_pool(name="p2ps", bufs=2, space="PSUM") as p2ps, \
             tc.tile_pool(name="p2ctx", bufs=2, space="PSUM") as p2ctx, \
             tc.tile_pool(name="p2dn", bufs=2, space="PSUM") as p2dn:
            for h in range(2):                   # local head; A2A #h after its 2 pairs
                for j in range(SQ_CHUNKS):
                    sq0 = j * 512
                    n_sk = 4 * (j + 1)
                    ctx_ps = {}
                    dn_ps = {}
                    for b in range(BATCH):
                        ctx_ps[b] = p2ctx.tile([128, 512], F32, name=f"ctx_ps{b}", tag="ctx_ps")
                        dn_ps[b] = p2dn.tile([1, 512], F32, name=f"dn_ps{b}", tag="dnbc")
                    prev_ex = {}
                    for g in range(n_sk // 2):       # groups of 2 sk-tiles
                        for b in range(BATCH):       # interleave the two batches
                            pair = h * 2 + b
                            sc_ps = p2ps.tile([128, 1024], F32, name=f"sc_ps{b}", tag="sc_ps")
                            for u in range(2):
                                i = 2 * g + u
                                nc.tensor.matmul(sc_ps[:, u * 512:(u + 1) * 512],
                                                 lhsT=k[:, pair, i * 128:(i + 1) * 128],
                                                 rhs=q[:, pair, sq0:sq0 + 512],
                                                 start=True, stop=True)
                            ex = p2exp.tile([128, 1024], BF16, name=f"ex{b}", tag="ex")
                            nc.scalar.activation(ex[:], sc_ps[:], EXP)
                            if 2 * g >= 4 * j:           # diagonal group: causal mask
                                m = 2 * g - 4 * j        # 0 or 2
                                nc.vector.tensor_mul(ex[:], ex[:], masks[:, m:m + 2, :]
                                                     .rearrange("p m f -> p (m f)"))
                            for u in range(2):
                                i = 2 * g + u
                                nc.tensor.matmul(ctx_ps[b][:], lhsT=v[:, pair, i, :],
                                                 rhs=ex[:, u * 512:(u + 1) * 512],
                                                 start=(i == 0), stop=(i == n_sk - 1))
                            # denominator: tree-sum 4 sk tiles in bf16, then one
                            # ones-matmul per quad into the f32 psum accumulator
                            t1 = p2dt.tile([128, 512], BF16, name=f"t1_{b}", tag="dtree")
                            nc.vector.tensor_add(t1[:], ex[:, 0:512], ex[:, 512:1024])
                            if g % 2 == 0:
                                prev_ex[b] = t1
                            else:
                                t3 = p2dt.tile([128, 512], BF16, name=f"t3_{b}", tag="dtree")
                                nc.vector.tensor_add(t3[:], t1[:], prev_ex[b][:])
                                nc.tensor.matmul(dn_ps[b][:], lhsT=ones128[:], rhs=t3[:],
                                                 start=(g == 1), stop=(g == n_sk // 2 - 1))
                    for b in range(BATCH):
                        # normalize: ctx * (1/denom) broadcast across partitions
                        recip = p2tmp.tile([1, 512], F32, name=f"recip{b}", tag="recip")
                        nc.vector.reciprocal_approx_fast(out=recip[:], in_=dn_ps[b][:])
                        recip16 = p2tmp.tile([1, 512], BF16, name=f"recip16{b}", tag="recip16")
                        nc.vector.tensor_copy(recip16[:], recip[:])
                        bc_ps = p2dn.tile([128, 512], F32, name=f"bc_ps{b}", tag="dnbc")
                        nc.tensor.matmul(bc_ps[:], lhsT=onesb[:], rhs=recip16[:],
                                         start=True, stop=True)
                        bc = p2tmp.tile([128, 512], F32, name=f"bc{b}", tag="bc")
                        nc.scalar.copy(bc[:], bc_ps[:])
                        ctx_sb = p2tmp.tile([128, 512], BF16, name=f"ctx_sb{b}", tag="ctx_sb")
                        nc.vector.tensor_mul(ctx_sb[:], ctx_ps[b][:], bc[:])
                        nc.sync.dma_start(out=a2a_in[h][4 * b + j, :, :], in_=ctx_sb[:])
                nc.gpsimd.collective_compute(
                    "AllToAll", mybir.AluOpType.bypass,
                    replica_groups=[list(range(N_CORES))],
                    ins=[a2a_in[h][:, :, :].opt()],
                    outs=[a2a_out[h][:, :, :].opt()])
                nc.sync.dma_start(out=ctxg[h][:],
                                  in_=a2a_out[h][:, :, :].rearrange("j p t -> p j t"))

        # ---- phase 3: output projection on this core's 512-token slice -----
        # split over the two A2A halves: even-head dims right after A2A#0
        # (overlaps A2A#1), odd-head dims after A2A#1.
        with tc.tile_pool(name="p3sb", bufs=3) as p3sb, \
             tc.tile_pool(name="p3half", bufs=16) as p3half, \
             tc.tile_pool(name="p3ps", bufs=3, space="PSUM") as p3ps:
            halves = {}
            for tt in range(4):
                t0 = tt * 128
                for fc in range(4):
                    f0 = fc * 512
                    ps = p3ps.tile([128, 512], F32, tag="o_ps")
                    nc.tensor.matmul(ps[:], lhsT=onesb[:], rhs=bo[:, f0:f0 + 512],
                                     start=True, stop=False)
                    for dt in range(8):
                        nc.tensor.matmul(ps[:], lhsT=ctxg[0][:, dt, t0:t0 + 128],
                                         rhs=wo[:, dt, f0:f0 + 512],
                                         start=False, stop=(dt == 7))
                    half = p3half.tile([128, 512], F32, tag="half")
                    nc.scalar.copy(half[:], ps[:])
                    halves[(tt, fc)] = half
            for tt in range(4):
                t0 = tt * 128
                for fc in range(4):
                    f0 = fc * 512
                    ps = p3ps.tile([128, 512], F32, tag="o_ps")
                    for dt in range(8, N_DT):
                        nc.tensor.matmul(ps[:], lhsT=ctxg[1][:, dt - 8, t0:t0 + 128],
                                         rhs=wo[:, dt, f0:f0 + 512],
                                         start=(dt == 8), stop=(dt == N_DT - 1))
                    osb = p3sb.tile([128, 512], F32, tag="osb")
                    nc.vector.tensor_add(osb[:], ps[:], halves[(tt, fc)][:])
                    nc.sync.dma_start(out=out_ext[t0:t0 + 128, f0:f0 + 512], in_=osb[:])


def _prep(x, freqs, W_qkv, b_qkv, W_o, b_o):
    """Host-side sharding/layout. Returns in_maps for the 8 cores."""
    perm = np.concatenate([np.arange(0, 128, 2), np.arange(1, 128, 2)])  # even dims first

    x_t = np.ascontiguousarray(x.transpose(2, 1, 0).reshape(D_MODEL, T)).astype(BF16_NP)

    cos = np.cos(freqs).astype(np.float32)       # [SEQ, 64]
    sin = np.sin(freqs).astype(np.float32)
    cosT = np.empty((2, 128, SEQ), np.float32)
    sinT = np.empty((2, 128, SEQ), np.float32)
    cosT[0, 0:64] = cos.T
    cosT[0, 64:128] = cos.T
    sinT[0, 0:64] = sin.T                        # bottom-half output uses +sin
    sinT[0, 64:128] = -sin.T                     # top-half output uses -sin
    scale = 1.0 / np.sqrt(np.float32(D_HEAD))
    cosT[1] = cosT[0] * scale
    sinT[1] = sinT[0] * scale
    cosT = cosT.astype(BF16_NP)
    sinT = sinT.astype(BF16_NP)

    m = np.empty((4, 128, 512), np.float32)
    p_idx = np.arange(128)[:, None]
    f_idx = np.arange(512)[None, :]
    for d in range(4):
        m[d] = (f_idx >= p_idx + 128 * d).astype(np.float32)
    masks = m.astype(BF16_NP)

    # W_o rows reordered: even global heads then odd (A2A #0 carries local head 0
    # of every core = even global heads)
    wo_order = np.concatenate([np.arange(N_HEADS)[::2], np.arange(N_HEADS)[1::2]])
    wo_t = np.ascontiguousarray(
        W_o.T.reshape(N_HEADS, D_HEAD, D_MODEL)[wo_order].reshape(D_MODEL, D_MODEL)
    ).astype(BF16_NP)
    bo = np.ascontiguousarray(b_o[None, :]).astype(BF16_NP)

    in_maps = []
    for c in range(N_CORES):
        rows = slice(256 * c, 256 * (c + 1))
        wq = W_qkv[0 * D_MODEL:1 * D_MODEL][rows].reshape(2, 128, D_MODEL)[:, perm]
        wk = W_qkv[1 * D_MODEL:2 * D_MODEL][rows].reshape(2, 128, D_MODEL)[:, perm]
        wv = W_qkv[2 * D_MODEL:3 * D_MODEL][rows]
        bq = b_qkv[0 * D_MODEL:1 * D_MODEL][rows].reshape(2, 128)[:, perm]
        bk = b_qkv[1 * D_MODEL:2 * D_MODEL][rows].reshape(2, 128)[:, perm]
        bv = b_qkv[2 * D_MODEL:3 * D_MODEL][rows]
        wqk = np.ascontiguousarray(
            np.concatenate([wq.reshape(256, D_MODEL), wk.reshape(256, D_MODEL)]).T
        ).astype(BF16_NP)
        wv_t = np.ascontiguousarray(wv.T).astype(BF16_NP)
        in_maps.append({
            "xt": x_t, "wqk": wqk,
            "bqk": np.ascontiguousarray(np.concatenate([bq, bk])).astype(np.float32),
            "wv": wv_t, "bv": np.ascontiguousarray(bv.reshape(2, 128)).astype(np.float32),
            "ident": np.eye(128, dtype=BF16_NP),
            "cosT": cosT, "sinT": sinT, "masks": masks,
            "wo": wo_t, "bo": bo,
        })
    return in_maps


def kernel(x, freqs, W_qkv, b_qkv, W_o, b_o, _trace=False, _tmpdir=None):
    from concourse.bass_utils import run_bass_kernel_spmd

    in_maps = _prep(np.asarray(x, np.float32), np.asarray(freqs, np.float32),
                    np.asarray(W_qkv, np.float32), np.asarray(b_qkv, np.float32),
                    np.asarray(W_o, np.float32), np.asarray(b_o, np.float32))
    if "nc" not in _cached:
        _cached["nc"] = build()
    res = run_bass_kernel_spmd(_cached["nc"], in_maps, core_ids=list(range(N_CORES)),
                               trace=_trace, tmpdir=_tmpdir)
    _cached["last_result"] = res
    full = np.concatenate([res.results[c]["out"] for c in range(N_CORES)], axis=0)
    # batch-major [T, D] -> (SEQ, BATCH, D)
    return np.ascontiguousarray(
        full.reshape(BATCH, SEQ, D_MODEL).transpose(1, 0, 2)).astype(np.float32)
